# revision 65
# baseline (speedup 1.0000x reference)
"""DiffVG-style circle renderer on 8 Trainium2 NeuronCores.

Final: per-core specialized programs via an 8-way tc.Switch on partition_id().
Each arm is fully static for its core: window widths/offsets are
compile-time constants, per-circle scalars (alpha, alpha*color) are
instruction immediates. This removes the shared cross-core max-width
padding (-25% elements), the dynamic-offset register machinery
(~140ns/slot on DVE), and the per-partition AP-scalar reads
(+60..150ns/op).

Per core: shard image by rows (128 rows/core); composite the circles
intersecting the band front-to-back (descending z) with transmittance T:
    w   = cov * T        (DVE TT 2x, cov = sigmoid(2(r-d)) from ACT/PSUM)
    T  += (-a) * w       (DVE stt, immediate scalar — chain stays on DVE)
    m_ch = (a col_ch) w  (premult, spread over ACT/Pool/DVE, immediates)
    C3  += m3            (DVE 3-plane TT, delayed 2 slots)
PE computes z=(r^2-d^2)/r per circle-pair as a K=8 outer-sum matmul with
bf16 hi/lo split operands. Output: [T->A, C_RGB] fp16 planes DMA'd out,
assembled to f32 on host.
"""

import sys

if "/opt/trn_rl_repo" not in sys.path:
    sys.path.insert(0, "/opt/trn_rl_repo")

import numpy as np
import ml_dtypes

import concourse.bass as bass
import concourse.bacc as bacc
import concourse.mybir as mybir
from concourse.tile import TileContext
from concourse import bass_utils

H = 1024
W = 1024
ROWS = 128
N_CORES = 8
MARGIN = 4.0
ROUND = 4
WMIN = 16
WCAP = 224
F32 = mybir.dt.float32
F16 = mybir.dt.float16
BF16 = mybir.dt.bfloat16
AF = mybir.ActivationFunctionType
OP = mybir.AluOpType
BF = ml_dtypes.bfloat16


# ---------------------------------------------------------------- host plan
def _core_circles(centers, radii, core):
    """Kept circle indices (descending z = front-to-back) + widths/offsets."""
    y0 = ROWS * core
    cy = centers[:, 1].astype(np.float64)
    cx = centers[:, 0].astype(np.float64)
    r = radii.astype(np.float64)
    keep = (cy + r + MARGIN >= y0 + 0.5) & (cy - r - MARGIN <= y0 + ROWS - 0.5)
    idx = np.where(keep)[0][::-1]  # descending index = front-to-back
    dymin = np.maximum(0.0, np.maximum(y0 + 0.5 - cy[idx],
                                       cy[idx] - (y0 + ROWS - 0.5)))
    rm = r[idx] + MARGIN
    halfw = np.sqrt(np.maximum(rm * rm - dymin * dymin, 4.0))
    ws = np.clip(np.ceil(2.0 * halfw / ROUND) * ROUND, WMIN, WCAP).astype(int)
    off = np.clip(np.round(cx[idx] - ws / 2.0), 0, W - ws).astype(int)
    return idx, ws, off


def _schedule(ws, off):
    """Topological reorder of the z-overlap DAG spacing each circle >=2
    slots after its last overlapping predecessor (deep-stack-first greedy).
    Returns (order, fo) where fo[k] = slot of k's first overlapping
    successor in the NEW order (n if none)."""
    n = len(ws)
    lo, hi = off, off + ws
    preds = [[] for _ in range(n)]
    succs = [[] for _ in range(n)]
    for u in range(n):
        for v in range(u + 1, n):
            if lo[u] < hi[v] and lo[v] < hi[u]:
                preds[v].append(u)
                succs[u].append(v)
    emitted_slot = {}
    emitted = np.zeros(n, bool)
    order = []
    for slot in range(n):
        ready = [v for v in range(n) if not emitted[v]
                 and all(emitted[u] for u in preds[v])]
        def lastpred(v):
            return max((emitted_slot[u] for u in preds[v]), default=-10)
        pool = ready
        for dist in (4, 3, 2):
            far = [v for v in ready if slot - lastpred(v) >= dist]
            if far:
                pool = far
                break
        if slot >= n - 12:
            v = min(pool, key=lambda v: ws[v])  # narrow tail: cheap taper
        else:
            v = max(pool, key=lambda v: (len([s for s in succs[v]
                                              if not emitted[s]]),
                                         -lastpred(v)))
        order.append(v)
        emitted[v] = True
        emitted_slot[v] = slot
    order = np.array(order, int)
    slot_of = np.empty(n, int)
    slot_of[order] = np.arange(n)
    fo = np.full(n, n, int)  # indexed by NEW slot
    lp = np.full(n, -10**6, int)  # last overlapping predecessor's slot
    root = np.zeros(n, bool)  # indexed by NEW slot
    for u in range(n):
        su = slot_of[u]
        if not preds[u]:
            root[su] = True
        for v in succs[u]:
            fo[su] = min(fo[su], slot_of[v])
            lp[slot_of[v]] = max(lp[slot_of[v]], su)
    return order, fo, root, lp


def make_plan(centers, radii, colors):
    """Per-core plan dicts with all compile-time constants."""
    col = colors.astype(np.float64)
    r = radii.astype(np.float64)
    cx = centers[:, 0].astype(np.float64)
    cy = centers[:, 1].astype(np.float64)
    plans = []
    for core in range(N_CORES):
        ids, ws, off = _core_circles(centers, radii, core)
        n = len(ids)
        order, fo, root, lp = _schedule(ws, off)
        ids, ws, off = ids[order], ws[order], off[order]
        npairs = (n + 1) // 2
        pairw = []
        for p in range(npairs):
            w0 = int(ws[2 * p])
            w1 = int(ws[2 * p + 1]) if 2 * p + 1 < n else 0
            pairw.append(w0 + w1)
        plans.append({
            "core": core, "ids": ids, "ws": ws.astype(int), "off": off,
            "n": n, "npairs": npairs, "pairw": pairw, "fo": fo, "root": root, "lp": lp,
            "alpha": col[ids, 3], "colr": col[ids, 0], "colg": col[ids, 1],
            "colb": col[ids, 2],
            "r": r[ids], "cx": cx[ids], "cy": cy[ids],
        })
    return plans


def _hilo(x):
    hi = x.astype(BF)
    lo = (x - hi.astype(np.float64)).astype(BF)
    return hi, lo


def make_inputs(plans):
    """rhs is group-strided: group g's 4 pairs are packed from DRAM offset
    g*1792 (pair_pos = within-group cumsum), so every core's group-g DMA
    reads the same DRAM range."""
    npairs_max = max(p["npairs"] for p in plans)
    ngroups_max = max((p["n"] + 7) // 8 for p in plans)
    # one DRAM tensor, group-strided: [4*ROWS lhsT chunk | 1792 rhs chunk]
    GSTR = 4 * ROWS + 1792
    ins = []
    for p in plans:
        core = p["core"]
        y0 = ROWS * core
        n = p["n"]
        op = np.zeros((8, ngroups_max * GSTR), BF)
        rows = y0 + np.arange(ROWS, dtype=np.float64) + 0.5
        for k in range(n):
            vk = int(p["ws"][k])
            off = int(p["off"][k])
            ri, cxi, cyi = p["r"][k], p["cx"][k], p["cy"][k]
            j = off + np.arange(vk, dtype=np.float64) + 0.5
            a = ri / 2.0 - (rows - cyi) ** 2 / ri
            b = ri / 2.0 - (j - cxi) ** 2 / ri
            ah, alo = _hilo(a)
            bh, blo = _hilo(b)
            pair, half = divmod(k, 2)
            rb = 4 * half
            g, ip = divmod(pair, 4)
            l0 = g * GSTR + ip * ROWS
            ls = slice(l0, l0 + ROWS)
            op[rb + 0, ls] = ah
            op[rb + 1, ls] = alo
            op[rb + 2, ls] = 1.0
            op[rb + 3, ls] = 1.0
            if g == 0:
                # group 0 sits on a fixed 224-grid (uniform across cores)
                # so its matmuls+sigmoids can run before the Switch
                c0 = g * GSTR + 4 * ROWS + (k - 8 * g) * WCAP
            else:
                c0 = g * GSTR + 4 * ROWS + sum(p["pairw"][4 * g:pair]) \
                    + (0 if half == 0 else int(p["ws"][2 * pair]))
            rs = slice(c0, c0 + vk)
            op[rb + 0, rs] = 1.0
            op[rb + 1, rs] = 1.0
            op[rb + 2, rs] = bh
            op[rb + 3, rs] = blo
        ins.append({"opnd": op})
    return ins, npairs_max, ngroups_max


# ------------------------------------------------- premult engine balancer
def _assign_premults(ws, pairw, fo, lp):
    """Greedy per-core assignment of each slot's 4 premult channels to ONE
    engine (ACT/Pool/DVE) — single producer per slot keeps the C4-add's
    cross-engine sync to one semaphore. Near slots (fused add due next
    slot) premultiply on DVE so the chain never waits cross-engine."""
    n = len(ws)
    dve = act = pool = 0.0
    for k in range(n):
        vk = float(ws[k])
        dve += (58 + vk / 2) / 0.96 + 80          # TT w'
        dve += (58 + vk) / 0.96 + 120             # stt imm (T update)
    for pw in pairw:
        act += (172 + pw) / 1.2 + 60              # sigmoid PSUM->SBUF
    c3_dve = lambda vk: (58 + 3 * vk / 2) / 0.96 + 120
    c3_pool = lambda vk: 3 * vk * 2.6 / 1.2 + 250
    cost = {
        "dve": lambda vk: 3 * ((58 + vk / 4) / 0.96 + 60),
        "act": lambda vk: 3 * ((224 + vk) / 1.2 + 60),
        "pool": lambda vk: 3 * (250 + vk * 1.2 / 1.2),
    }
    busy = {"dve": dve, "act": act, "pool": pool}
    assign = []
    for k in range(n):
        vk = float(ws[k])
        if k >= n - 12:
            e = "dve"  # tail-taper: let ACT/Pool drain
        else:
            e = min(busy, key=lambda e: (busy[e] + cost[e](vk))
                    * (1.35 if e == "dve" else 1.0))
        busy[e] += cost[e](vk)
        # C3-add: keep on DVE unless this is a Pool slot and Pool+C3 is
        # projected cheaper than loading DVE further (zero cross-engine
        # sync since the premults are already there)
        isolated = (fo[k] - k >= 8) and (k - lp[k] >= 8)
        if (e == "pool" and isolated and k < n - 14
                and busy["pool"] + c3_pool(vk) < busy["dve"] + c3_dve(vk)):
            busy["pool"] += 300.0  # SWDGE descgen on the Pool queue
            c3e = "dma"
        else:
            busy["dve"] += c3_dve(vk)
            c3e = "dve"
        assign.append((e, c3e))
    return assign


# ------------------------------------------------------------- device build
def build_nc(plans, npairs_max, ngroups_max):
    GSTR = 4 * ROWS + 1792
    nc = bacc.Bacc("TRN2", target_bir_lowering=False, debug=False,
                   num_devices=N_CORES)
    opnd_d = nc.dram_tensor("opnd", [8, ngroups_max * GSTR], BF16,
                            kind="ExternalInput").ap()
    out_d = nc.dram_tensor("out", [ROWS, 4 * W], F16,
                           kind="ExternalOutput").ap()

    with TileContext(nc) as tc:
        # TCC: plane 0 = transmittance T, planes 1-3 = premultiplied RGB.
        # One 4-plane TT add per slot applies both the T update (m_A = -a*w)
        # and the color accumulation (m_ch = a*col_ch*w).
        TCC = nc.alloc_sbuf_tensor("TCC", [ROWS, 4 * W], F16).ap()
        AT = nc.alloc_sbuf_tensor("AT", [ROWS, W], F16).ap()
        covr = nc.alloc_sbuf_tensor("covr", [ROWS, 3 * 1792], F16).ap()
        wr = nc.alloc_sbuf_tensor("wr", [ROWS, 8 * WCAP], F16).ap()
        mr = nc.alloc_sbuf_tensor("mr", [ROWS, 8 * 3 * WCAP], F16).ap()
        op_sb = nc.alloc_sbuf_tensor("op_sb", [8, 2 * GSTR], BF16).ap()
        pt0 = nc.alloc_psum_tensor("pt0", [ROWS, 4 * 512], F32).ap()
        pt1 = nc.alloc_psum_tensor("pt1", [ROWS, 4 * 512], F32).ap()
        pts = [pt0, pt1]

        # prefetch the first two groups' operands FIRST: the transfer takes
        # ~3.5us end-to-end and the first matmul is gated on it, so its
        # issue must not queue behind the pid-load/dispatch machinery
        cut = 4 * ROWS + 448  # pair-0 lhsT + rhs: first matmul's gate
        nc.sync.dma_start(op_sb[:, 0:cut], opnd_d[:, 0:cut])
        nc.sync.dma_start(op_sb[:, cut:GSTR], opnd_d[:, cut:GSTR])
        if ngroups_max > 1:
            nc.sync.dma_start(op_sb[:, GSTR:2 * GSTR],
                              opnd_d[:, GSTR:2 * GSTR])

        pid = nc.partition_id()
        from concourse.expressions import s_valid_engines
        hint = tc.switch_hint({e: pid for e in s_valid_engines(pid)},
                              N_CORES, label="corearm")

        T = TCC[:, 0:W]
        nc.gpsimd.memset(T, 1.0)
        nc.gpsimd.memset(TCC[:, W:4 * W], 0.0)

        TCC4 = TCC.rearrange("p (c x) -> p c x", x=W)
        CC3 = TCC[:, W:4 * W].rearrange("p (c x) -> p c x", x=W)
        mr3 = mr.rearrange("p (s x) -> p s x", x=WCAP)

        # groups 0-1 are on a fixed 224-grid on every core: run their
        # matmuls and sigmoids before the Switch, overlapping the arm
        # I-cache fill
        for g in (0,):
            h = g % 2
            for i in range(4):
                # alternate PSUM buffers so matmul i+1 never waits on
                # sigmoid i (PSUM deps track coarser than 512-col ranges)
                pb = pts[i % 2][:, (i // 2) * 1024:(i // 2) * 1024 + 448]
                nc.tensor.matmul(
                    pb,
                    op_sb[:, h * GSTR + i * ROWS:h * GSTR + (i + 1) * ROWS],
                    op_sb[:, h * GSTR + 4 * ROWS + i * 448:
                          h * GSTR + 4 * ROWS + (i + 1) * 448],
                    start=True, stop=True)
                if i == 0:
                    # slot-0 half first: the first chain op only needs it
                    nc.scalar.activation(covr[:, 0:224], pb[:, 0:224],
                                         AF.Sigmoid)
                    nc.scalar.activation(covr[:, 224:448], pb[:, 224:448],
                                         AF.Sigmoid)
                else:
                    nc.scalar.activation(
                        covr[:, (g % 3) * 1792 + i * 448:
                             (g % 3) * 1792 + (i + 1) * 448],
                        pb, AF.Sigmoid)

        for core in tc.Switch(pid, N_CORES, hint=hint):
            p = plans[core]
            n = p["n"]
            ws, off = p["ws"], p["off"]
            pairw = p["pairw"]
            fo = p["fo"]
            assign = _assign_premults(ws, pairw, fo, p["lp"])
            # C3-add for slot k can lag up to 6 slots (mr ring depth 8) —
            # only the stt updates T, so C adds never gate the chain.
            place = {}
            for k in range(n):
                j = min(k + 6, n)
                place.setdefault(j, []).append(k)

            def emit_c3(k):
                vk = int(ws[k])
                ok = int(off[k])
                qv = CC3[:, :, ok:ok + vk]
                mp = mr3[:, (k % 8) * 3:(k % 8) * 3 + 3, :vk]
                if assign[k][1] == "dma":
                    nc.gpsimd.dma_start(qv, mp, accum_op=OP.add)
                else:
                    nc.vector.tensor_tensor(qv, qv, mp, OP.add)

            stt_done = set()
            wsrc = {}

            def emit_stt(k):
                if k < 0 or k in stt_done:
                    return
                stt_done.add(k)
                vq, oq = int(ws[k]), int(off[k])
                nc.vector.scalar_tensor_tensor(
                    T[:, oq:oq + vq], wsrc[k],
                    -float(p["alpha"][k]), T[:, oq:oq + vq],
                    OP.mult, OP.add)

            ngroups = (n + 7) // 8
            for g in range(ngroups):
                k0 = g * 8
                gsize = min(8, n - k0)
                gp = (gsize + 1) // 2
                p0 = k0 // 2
                lh = op_sb[:, (g % 2) * GSTR:]
                rh = op_sb[:, (g % 2) * GSTR + 4 * ROWS:]
                if g >= 2:
                    nc.sync.dma_start(
                        op_sb[:, (g % 2) * GSTR:(g % 2 + 1) * GSTR],
                        opnd_d[:, g * GSTR:(g + 1) * GSTR])
                pt = pts[g % 2]
                cbase = (g % 3) * 1792
                rpos = 0
                pair_pos = []
                if g > 0:
                    for i in range(gp):
                        pw = pairw[p0 + i]
                        pair_pos.append(rpos)
                        nc.tensor.matmul(
                            pt[:, i * 512:i * 512 + pw],
                            lh[:, i * ROWS:(i + 1) * ROWS],
                            rh[:, rpos:rpos + pw],
                            start=True, stop=True)
                        rpos += pw

                rpos = 0
                for j in range(gsize):
                    k = k0 + j
                    vk = int(ws[k])
                    ok = int(off[k])
                    if g == 0:
                        rpos = j * WCAP  # fixed grid, sigmoid pre-dispatch
                    elif j % 2 == 0:
                        i = j // 2
                        pw = pairw[p0 + i]
                        nc.scalar.activation(
                            covr[:, cbase + pair_pos[i]:
                                 cbase + pair_pos[i] + pw],
                            pt[:, i * 512:i * 512 + pw], AF.Sigmoid)
                    cov = covr[:, cbase + rpos:cbase + rpos + vk]
                    rpos += vk
                    wv = wr[:, (k % 8) * WCAP:(k % 8) * WCAP + vk]
                    tw = T[:, ok:ok + vk]
                    al = float(p["alpha"][k])
                    acr = float(p["alpha"][k] * p["colr"][k])
                    acg = float(p["alpha"][k] * p["colg"][k])
                    acb = float(p["alpha"][k] * p["colb"][k])
                    # DVE: w = cov*T, then lagged C add, then the PREVIOUS
                    # slot's T update — the one-slot stt lag keeps every
                    # same-window DVE dependency >=3 ops apart so Tile's
                    # same-engine completion waits (~600ns) never fire.
                    # Safe: the scheduler spaces overlapping windows >=2
                    # slots, so slot k+1's w' never needs slot k's update.
                    # Root slots (no overlapping higher-z circle) see T==1:
                    # w == cov, so the w'-TT is skipped entirely.
                    if p["root"][k]:
                        wsrc[k] = cov
                    else:
                        wsrc[k] = wv
                        nc.vector.tensor_tensor(wv, cov, tw, OP.mult)
                    for kp in place.get(k, ()):
                        emit_c3(kp)
                    emit_stt(k - 1)
                    if int(fo[k]) <= k + 1:
                        # next slot overlaps: T update cannot be delayed
                        emit_stt(k)
                    # premults m_ch = (a col_ch) * w, one engine per slot
                    eng = assign[k][0]
                    wk = wsrc[k]
                    for ch, sc in enumerate((acr, acg, acb)):
                        mb = ((k % 8) * 3 + ch) * WCAP
                        if eng == "dve":
                            nc.vector.tensor_scalar(
                                mr[:, mb:mb + vk], wk, sc, 0.0,
                                OP.mult, OP.add)
                        elif eng == "pool":
                            nc.gpsimd.tensor_scalar(
                                mr[:, mb:mb + vk], wk, sc, 0.0,
                                OP.mult, OP.add)
                        else:
                            nc.scalar.activation(
                                mr[:, mb:mb + vk], wk, AF.Copy, scale=sc)
            # only slot n-1's stt is pending: compute A everywhere else now
            alo, ahi = int(off[n - 1]), int(off[n - 1]) + int(ws[n - 1])
            if alo > 0:
                nc.scalar.activation(AT[:, 0:alo], T[:, 0:alo], AF.Copy,
                                     bias=1.0, scale=-1.0)
                nc.sync.dma_start(out_d[:, 3 * W:3 * W + alo], AT[:, 0:alo])
            if ahi < W:
                nc.scalar.activation(AT[:, ahi:W], T[:, ahi:W], AF.Copy,
                                     bias=1.0, scale=-1.0)
                nc.sync.dma_start(out_d[:, 3 * W + ahi:4 * W], AT[:, ahi:W])
            emit_stt(n - 1)
            nc.scalar.activation(AT[:, alo:ahi], T[:, alo:ahi], AF.Copy,
                                 bias=1.0, scale=-1.0)
            nc.sync.dma_start(out_d[:, 3 * W + alo:3 * W + ahi],
                              AT[:, alo:ahi])
            flushks = sorted(kk for j, ks in place.items() if j >= n
                             for kk in ks)
            xlo = min((int(off[k]) for k in flushks), default=0)
            xhi = max((int(off[k]) + int(ws[k]) for k in flushks), default=0)
            out3 = out_d[:, 0:3 * W].rearrange("p (c x) -> p c x", x=W)
            src3 = TCC[:, W:4 * W].rearrange("p (c x) -> p c x", x=W)
            if xlo > 0:
                nc.sync.dma_start(out3[:, :, 0:xlo], src3[:, :, 0:xlo])
            if xhi < W:
                nc.sync.dma_start(out3[:, :, xhi:W], src3[:, :, xhi:W])
            for k in flushks:
                emit_c3(k)
            if xhi > xlo:
                nc.sync.dma_start(out3[:, :, xlo:xhi], src3[:, :, xlo:xhi])


    nc.compile()
    return nc


def kernel(centers, radii, colors):
    centers = np.asarray(centers, np.float32)
    radii = np.asarray(radii, np.float32)
    colors = np.asarray(colors, np.float32)

    plans = make_plan(centers, radii, colors)
    ins, npairs_max, ngroups_max = make_inputs(plans)
    nc = build_nc(plans, npairs_max, ngroups_max)
    res = bass_utils.run_bass_kernel_spmd(nc, ins, list(range(N_CORES)),
                                          trace=False)
    out = np.empty((H, W, 4), np.float32)
    for c in range(N_CORES):
        planes = res.results[c]["out"].astype(np.float32)  # [128, 4*W]
        for ch in range(4):
            out[c * ROWS:(c + 1) * ROWS, :, ch] = planes[:, ch * W:(ch + 1) * W]
    return out


# revision 66
# speedup vs baseline: 1.0333x; 1.0333x over previous
"""DiffVG-style circle renderer on 8 Trainium2 NeuronCores.

Final: per-core specialized programs via an 8-way tc.Switch on partition_id().
Each arm is fully static for its core: window widths/offsets are
compile-time constants, per-circle scalars (alpha, alpha*color) are
instruction immediates. This removes the shared cross-core max-width
padding (-25% elements), the dynamic-offset register machinery
(~140ns/slot on DVE), and the per-partition AP-scalar reads
(+60..150ns/op).

Per core: shard image by rows (128 rows/core); composite the circles
intersecting the band front-to-back (descending z) with transmittance T:
    w   = cov * T        (DVE TT 2x, cov = sigmoid(2(r-d)) from ACT/PSUM)
    T  += (-a) * w       (DVE stt, immediate scalar — chain stays on DVE)
    m_ch = (a col_ch) w  (premult, spread over ACT/Pool/DVE, immediates)
    C3  += m3            (DVE 3-plane TT, delayed 2 slots)
PE computes z=(r^2-d^2)/r per circle-pair as a K=8 outer-sum matmul with
bf16 hi/lo split operands. Output: [T->A, C_RGB] fp16 planes DMA'd out,
assembled to f32 on host.
"""

import sys

if "/opt/trn_rl_repo" not in sys.path:
    sys.path.insert(0, "/opt/trn_rl_repo")

import numpy as np
import ml_dtypes

import concourse.bass as bass
import concourse.bacc as bacc
import concourse.mybir as mybir
from concourse.tile import TileContext
from concourse import bass_utils

H = 1024
W = 1024
ROWS = 128
N_CORES = 8
MARGIN = 4.0
ROUND = 4
WMIN = 16
WCAP = 224
F32 = mybir.dt.float32
F16 = mybir.dt.float16
BF16 = mybir.dt.bfloat16
AF = mybir.ActivationFunctionType
OP = mybir.AluOpType
BF = ml_dtypes.bfloat16


# ---------------------------------------------------------------- host plan
def _core_circles(centers, radii, core):
    """Kept circle indices (descending z = front-to-back) + widths/offsets."""
    y0 = ROWS * core
    cy = centers[:, 1].astype(np.float64)
    cx = centers[:, 0].astype(np.float64)
    r = radii.astype(np.float64)
    keep = (cy + r + MARGIN >= y0 + 0.5) & (cy - r - MARGIN <= y0 + ROWS - 0.5)
    idx = np.where(keep)[0][::-1]  # descending index = front-to-back
    dymin = np.maximum(0.0, np.maximum(y0 + 0.5 - cy[idx],
                                       cy[idx] - (y0 + ROWS - 0.5)))
    rm = r[idx] + MARGIN
    halfw = np.sqrt(np.maximum(rm * rm - dymin * dymin, 4.0))
    ws = np.clip(np.ceil(2.0 * halfw / ROUND) * ROUND, WMIN, WCAP).astype(int)
    off = np.clip(np.round(cx[idx] - ws / 2.0), 0, W - ws).astype(int)
    return idx, ws, off


def _schedule(ws, off):
    """Topological reorder of the z-overlap DAG spacing each circle >=2
    slots after its last overlapping predecessor (deep-stack-first greedy).
    Returns (order, fo) where fo[k] = slot of k's first overlapping
    successor in the NEW order (n if none)."""
    n = len(ws)
    lo, hi = off, off + ws
    preds = [[] for _ in range(n)]
    succs = [[] for _ in range(n)]
    for u in range(n):
        for v in range(u + 1, n):
            if lo[u] < hi[v] and lo[v] < hi[u]:
                preds[v].append(u)
                succs[u].append(v)
    emitted_slot = {}
    emitted = np.zeros(n, bool)
    order = []
    for slot in range(n):
        ready = [v for v in range(n) if not emitted[v]
                 and all(emitted[u] for u in preds[v])]
        def lastpred(v):
            return max((emitted_slot[u] for u in preds[v]), default=-10)
        pool = ready
        for dist in (4, 3, 2):
            far = [v for v in ready if slot - lastpred(v) >= dist]
            if far:
                pool = far
                break
        if slot >= n - 12:
            v = min(pool, key=lambda v: ws[v])  # narrow tail: cheap taper
        else:
            v = max(pool, key=lambda v: (len([s for s in succs[v]
                                              if not emitted[s]]),
                                         -lastpred(v)))
        order.append(v)
        emitted[v] = True
        emitted_slot[v] = slot
    order = np.array(order, int)
    slot_of = np.empty(n, int)
    slot_of[order] = np.arange(n)
    fo = np.full(n, n, int)  # indexed by NEW slot
    lp = np.full(n, -10**6, int)  # last overlapping predecessor's slot
    root = np.zeros(n, bool)  # indexed by NEW slot
    for u in range(n):
        su = slot_of[u]
        if not preds[u]:
            root[su] = True
        for v in succs[u]:
            fo[su] = min(fo[su], slot_of[v])
            lp[slot_of[v]] = max(lp[slot_of[v]], su)
    return order, fo, root, lp


def make_plan(centers, radii, colors):
    """Per-core plan dicts with all compile-time constants."""
    col = colors.astype(np.float64)
    r = radii.astype(np.float64)
    cx = centers[:, 0].astype(np.float64)
    cy = centers[:, 1].astype(np.float64)
    plans = []
    for core in range(N_CORES):
        ids, ws, off = _core_circles(centers, radii, core)
        n = len(ids)
        order, fo, root, lp = _schedule(ws, off)
        ids, ws, off = ids[order], ws[order], off[order]
        npairs = (n + 1) // 2
        pairw = []
        for p in range(npairs):
            w0 = int(ws[2 * p])
            w1 = int(ws[2 * p + 1]) if 2 * p + 1 < n else 0
            pairw.append(w0 + w1)
        plans.append({
            "core": core, "ids": ids, "ws": ws.astype(int), "off": off,
            "n": n, "npairs": npairs, "pairw": pairw, "fo": fo, "root": root, "lp": lp,
            "alpha": col[ids, 3], "colr": col[ids, 0], "colg": col[ids, 1],
            "colb": col[ids, 2],
            "r": r[ids], "cx": cx[ids], "cy": cy[ids],
        })
    return plans


def _hilo(x):
    hi = x.astype(BF)
    lo = (x - hi.astype(np.float64)).astype(BF)
    return hi, lo


def make_inputs(plans):
    """rhs is group-strided: group g's 4 pairs are packed from DRAM offset
    g*1792 (pair_pos = within-group cumsum), so every core's group-g DMA
    reads the same DRAM range."""
    npairs_max = max(p["npairs"] for p in plans)
    ngroups_max = max((p["n"] + 7) // 8 for p in plans)
    # one DRAM tensor, group-strided: [4*ROWS lhsT chunk | 1792 rhs chunk]
    GSTR = 4 * ROWS + 1792
    ins = []
    for p in plans:
        core = p["core"]
        y0 = ROWS * core
        n = p["n"]
        op = np.zeros((8, ngroups_max * GSTR), BF)
        rows = y0 + np.arange(ROWS, dtype=np.float64) + 0.5
        for k in range(n):
            vk = int(p["ws"][k])
            off = int(p["off"][k])
            ri, cxi, cyi = p["r"][k], p["cx"][k], p["cy"][k]
            j = off + np.arange(vk, dtype=np.float64) + 0.5
            a = ri / 2.0 - (rows - cyi) ** 2 / ri
            b = ri / 2.0 - (j - cxi) ** 2 / ri
            ah, alo = _hilo(a)
            bh, blo = _hilo(b)
            pair, half = divmod(k, 2)
            rb = 4 * half
            g, ip = divmod(pair, 4)
            l0 = g * GSTR + ip * ROWS
            ls = slice(l0, l0 + ROWS)
            op[rb + 0, ls] = ah
            op[rb + 1, ls] = alo
            op[rb + 2, ls] = 1.0
            op[rb + 3, ls] = 1.0
            if g == 0:
                # group 0 sits on a fixed 224-grid (uniform across cores)
                # so its matmuls+sigmoids can run before the Switch
                c0 = g * GSTR + 4 * ROWS + (k - 8 * g) * WCAP
            else:
                c0 = g * GSTR + 4 * ROWS + sum(p["pairw"][4 * g:pair]) \
                    + (0 if half == 0 else int(p["ws"][2 * pair]))
            rs = slice(c0, c0 + vk)
            op[rb + 0, rs] = 1.0
            op[rb + 1, rs] = 1.0
            op[rb + 2, rs] = bh
            op[rb + 3, rs] = blo
        ins.append({"opnd": op})
    return ins, npairs_max, ngroups_max


# ------------------------------------------------- premult engine balancer
def _assign_premults(ws, pairw, fo, lp):
    """Greedy per-core assignment of each slot's 4 premult channels to ONE
    engine (ACT/Pool/DVE) — single producer per slot keeps the C4-add's
    cross-engine sync to one semaphore. Near slots (fused add due next
    slot) premultiply on DVE so the chain never waits cross-engine."""
    n = len(ws)
    dve = act = pool = 0.0
    for k in range(n):
        vk = float(ws[k])
        dve += (58 + vk / 2) / 0.96 + 80          # TT w'
        dve += (58 + vk) / 0.96 + 120             # stt imm (T update)
    for pw in pairw:
        act += (172 + pw) / 1.2 + 60              # sigmoid PSUM->SBUF
    c3_dve = lambda vk: (58 + 3 * vk / 2) / 0.96 + 120
    c3_pool = lambda vk: 3 * vk * 2.6 / 1.2 + 250
    cost = {
        "dve": lambda vk: 3 * ((58 + vk / 4) / 0.96 + 60),
        "act": lambda vk: 3 * ((224 + vk) / 1.2 + 60),
        "pool": lambda vk: 3 * (250 + vk * 1.2 / 1.2),
    }
    busy = {"dve": dve, "act": act, "pool": pool}
    assign = []
    for k in range(n):
        vk = float(ws[k])
        if k >= n - 12:
            e = "dve"  # tail-taper: let ACT/Pool drain
        else:
            e = min(busy, key=lambda e: (busy[e] + cost[e](vk))
                    * (1.35 if e == "dve" else 1.0))
        busy[e] += cost[e](vk)
        # C3-add: keep on DVE unless this is a Pool slot and Pool+C3 is
        # projected cheaper than loading DVE further (zero cross-engine
        # sync since the premults are already there)
        isolated = (fo[k] - k >= 8) and (k - lp[k] >= 8)
        if (e == "pool" and isolated and k < n - 14
                and busy["pool"] + c3_pool(vk) < busy["dve"] + c3_dve(vk)):
            busy["pool"] += 300.0  # SWDGE descgen on the Pool queue
            c3e = "dma"
        else:
            busy["dve"] += c3_dve(vk)
            c3e = "dve"
        assign.append((e, c3e))
    return assign


# ------------------------------------------------------------- device build
def build_nc(plans, npairs_max, ngroups_max):
    GSTR = 4 * ROWS + 1792
    nc = bacc.Bacc("TRN2", target_bir_lowering=False, debug=False,
                   num_devices=N_CORES)
    opnd_d = nc.dram_tensor("opnd", [8, ngroups_max * GSTR], BF16,
                            kind="ExternalInput").ap()
    out_d = nc.dram_tensor("out", [ROWS, 4 * W], F16,
                           kind="ExternalOutput").ap()

    with TileContext(nc) as tc:
        # TCC: plane 0 = transmittance T, planes 1-3 = premultiplied RGB.
        # One 4-plane TT add per slot applies both the T update (m_A = -a*w)
        # and the color accumulation (m_ch = a*col_ch*w).
        TCC = nc.alloc_sbuf_tensor("TCC", [ROWS, 4 * W], F16).ap()
        AT = nc.alloc_sbuf_tensor("AT", [ROWS, W], F16).ap()
        covr = nc.alloc_sbuf_tensor("covr", [ROWS, 3 * 1792], F16).ap()
        wr = nc.alloc_sbuf_tensor("wr", [ROWS, 8 * WCAP], F16).ap()
        mr = nc.alloc_sbuf_tensor("mr", [ROWS, 8 * 3 * WCAP], F16).ap()
        op_sb = nc.alloc_sbuf_tensor("op_sb", [8, 2 * GSTR], BF16).ap()
        pt0 = nc.alloc_psum_tensor("pt0", [ROWS, 4 * 512], F32).ap()
        pt1 = nc.alloc_psum_tensor("pt1", [ROWS, 4 * 512], F32).ap()
        pts = [pt0, pt1]

        # prefetch the first two groups' operands FIRST: the transfer takes
        # ~3.5us end-to-end and the first matmul is gated on it, so its
        # issue must not queue behind the pid-load/dispatch machinery
        cut = 4 * ROWS + 448  # pair-0 lhsT + rhs: first matmul's gate
        nc.sync.dma_start(op_sb[:, 0:cut], opnd_d[:, 0:cut])
        nc.sync.dma_start(op_sb[:, cut:GSTR], opnd_d[:, cut:GSTR])
        if ngroups_max > 1:
            nc.sync.dma_start(op_sb[:, GSTR:2 * GSTR],
                              opnd_d[:, GSTR:2 * GSTR])

        pid = nc.partition_id()
        from concourse.expressions import s_valid_engines
        hint = tc.switch_hint({e: pid for e in s_valid_engines(pid)},
                              N_CORES, label="corearm")

        T = TCC[:, 0:W]
        nc.gpsimd.memset(T, 1.0)
        nc.gpsimd.memset(TCC[:, W:4 * W], 0.0)

        TCC4 = TCC.rearrange("p (c x) -> p c x", x=W)
        CC3 = TCC[:, W:4 * W].rearrange("p (c x) -> p c x", x=W)
        mr3 = mr.rearrange("p (s x) -> p s x", x=WCAP)

        # groups 0-1 are on a fixed 224-grid on every core: run their
        # matmuls and sigmoids before the Switch, overlapping the arm
        # I-cache fill
        for g in (0,):
            h = g % 2
            for i in range(4):
                # alternate PSUM buffers so matmul i+1 never waits on
                # sigmoid i (PSUM deps track coarser than 512-col ranges)
                pb = pts[i % 2][:, (i // 2) * 1024:(i // 2) * 1024 + 448]
                nc.tensor.matmul(
                    pb,
                    op_sb[:, h * GSTR + i * ROWS:h * GSTR + (i + 1) * ROWS],
                    op_sb[:, h * GSTR + 4 * ROWS + i * 448:
                          h * GSTR + 4 * ROWS + (i + 1) * 448],
                    start=True, stop=True)
                nc.scalar.activation(
                    covr[:, (g % 3) * 1792 + i * 448:
                         (g % 3) * 1792 + (i + 1) * 448],
                    pb, AF.Sigmoid)

        for core in tc.Switch(pid, N_CORES, hint=hint):
            p = plans[core]
            n = p["n"]
            ws, off = p["ws"], p["off"]
            pairw = p["pairw"]
            fo = p["fo"]
            assign = _assign_premults(ws, pairw, fo, p["lp"])
            # C3-add for slot k can lag up to 6 slots (mr ring depth 8) —
            # only the stt updates T, so C adds never gate the chain.
            place = {}
            for k in range(n):
                j = min(k + 6, n)
                place.setdefault(j, []).append(k)

            def emit_c3(k):
                vk = int(ws[k])
                ok = int(off[k])
                qv = CC3[:, :, ok:ok + vk]
                mp = mr3[:, (k % 8) * 3:(k % 8) * 3 + 3, :vk]
                if assign[k][1] == "dma":
                    nc.gpsimd.dma_start(qv, mp, accum_op=OP.add)
                else:
                    nc.vector.tensor_tensor(qv, qv, mp, OP.add)

            stt_done = set()
            wsrc = {}

            def emit_stt(k):
                if k < 0 or k in stt_done:
                    return
                stt_done.add(k)
                vq, oq = int(ws[k]), int(off[k])
                nc.vector.scalar_tensor_tensor(
                    T[:, oq:oq + vq], wsrc[k],
                    -float(p["alpha"][k]), T[:, oq:oq + vq],
                    OP.mult, OP.add)

            ngroups = (n + 7) // 8
            for g in range(ngroups):
                k0 = g * 8
                gsize = min(8, n - k0)
                gp = (gsize + 1) // 2
                p0 = k0 // 2
                lh = op_sb[:, (g % 2) * GSTR:]
                rh = op_sb[:, (g % 2) * GSTR + 4 * ROWS:]
                if g >= 2:
                    nc.sync.dma_start(
                        op_sb[:, (g % 2) * GSTR:(g % 2 + 1) * GSTR],
                        opnd_d[:, g * GSTR:(g + 1) * GSTR])
                pt = pts[g % 2]
                cbase = (g % 3) * 1792
                rpos = 0
                pair_pos = []
                if g > 0:
                    for i in range(gp):
                        pw = pairw[p0 + i]
                        pair_pos.append(rpos)
                        nc.tensor.matmul(
                            pt[:, i * 512:i * 512 + pw],
                            lh[:, i * ROWS:(i + 1) * ROWS],
                            rh[:, rpos:rpos + pw],
                            start=True, stop=True)
                        rpos += pw

                rpos = 0
                for j in range(gsize):
                    k = k0 + j
                    vk = int(ws[k])
                    ok = int(off[k])
                    if g == 0:
                        rpos = j * WCAP  # fixed grid, sigmoid pre-dispatch
                    elif j % 2 == 0:
                        i = j // 2
                        pw = pairw[p0 + i]
                        nc.scalar.activation(
                            covr[:, cbase + pair_pos[i]:
                                 cbase + pair_pos[i] + pw],
                            pt[:, i * 512:i * 512 + pw], AF.Sigmoid)
                    cov = covr[:, cbase + rpos:cbase + rpos + vk]
                    rpos += vk
                    wv = wr[:, (k % 8) * WCAP:(k % 8) * WCAP + vk]
                    tw = T[:, ok:ok + vk]
                    al = float(p["alpha"][k])
                    acr = float(p["alpha"][k] * p["colr"][k])
                    acg = float(p["alpha"][k] * p["colg"][k])
                    acb = float(p["alpha"][k] * p["colb"][k])
                    # DVE: w = cov*T, then lagged C add, then the PREVIOUS
                    # slot's T update — the one-slot stt lag keeps every
                    # same-window DVE dependency >=3 ops apart so Tile's
                    # same-engine completion waits (~600ns) never fire.
                    # Safe: the scheduler spaces overlapping windows >=2
                    # slots, so slot k+1's w' never needs slot k's update.
                    # Root slots (no overlapping higher-z circle) see T==1:
                    # w == cov, so the w'-TT is skipped entirely.
                    if p["root"][k]:
                        wsrc[k] = cov
                    else:
                        wsrc[k] = wv
                        nc.vector.tensor_tensor(wv, cov, tw, OP.mult)
                    for kp in place.get(k, ()):
                        emit_c3(kp)
                    emit_stt(k - 1)
                    if int(fo[k]) <= k + 1:
                        # next slot overlaps: T update cannot be delayed
                        emit_stt(k)
                    # premults m_ch = (a col_ch) * w, one engine per slot
                    eng = assign[k][0]
                    wk = wsrc[k]
                    for ch, sc in enumerate((acr, acg, acb)):
                        mb = ((k % 8) * 3 + ch) * WCAP
                        if eng == "dve":
                            nc.vector.tensor_scalar(
                                mr[:, mb:mb + vk], wk, sc, 0.0,
                                OP.mult, OP.add)
                        elif eng == "pool":
                            nc.gpsimd.tensor_scalar(
                                mr[:, mb:mb + vk], wk, sc, 0.0,
                                OP.mult, OP.add)
                        else:
                            nc.scalar.activation(
                                mr[:, mb:mb + vk], wk, AF.Copy, scale=sc)
            emit_stt(n - 1)
            # A = 1 - T on ACT (Copy with scale=-1, bias=1) frees DVE's tail
            nc.scalar.activation(AT, T, AF.Copy, bias=1.0, scale=-1.0)
            nc.sync.dma_start(out_d[:, 3 * W:4 * W], AT)
            flushks = sorted(kk for j, ks in place.items() if j >= n
                             for kk in ks)
            xlo = min((int(off[k]) for k in flushks), default=0)
            xhi = max((int(off[k]) + int(ws[k]) for k in flushks), default=0)
            out3 = out_d[:, 0:3 * W].rearrange("p (c x) -> p c x", x=W)
            src3 = TCC[:, W:4 * W].rearrange("p (c x) -> p c x", x=W)
            if xlo > 0:
                nc.sync.dma_start(out3[:, :, 0:xlo], src3[:, :, 0:xlo])
            if xhi < W:
                nc.sync.dma_start(out3[:, :, xhi:W], src3[:, :, xhi:W])
            for k in flushks:
                emit_c3(k)
            if xhi > xlo:
                nc.sync.dma_start(out3[:, :, xlo:xhi], src3[:, :, xlo:xhi])


    nc.compile()
    return nc


def kernel(centers, radii, colors):
    centers = np.asarray(centers, np.float32)
    radii = np.asarray(radii, np.float32)
    colors = np.asarray(colors, np.float32)

    plans = make_plan(centers, radii, colors)
    ins, npairs_max, ngroups_max = make_inputs(plans)
    nc = build_nc(plans, npairs_max, ngroups_max)
    res = bass_utils.run_bass_kernel_spmd(nc, ins, list(range(N_CORES)),
                                          trace=False)
    out = np.empty((H, W, 4), np.float32)
    for c in range(N_CORES):
        planes = res.results[c]["out"].astype(np.float32)  # [128, 4*W]
        for ch in range(4):
            out[c * ROWS:(c + 1) * ROWS, :, ch] = planes[:, ch * W:(ch + 1) * W]
    return out


# revision 67
# speedup vs baseline: 1.0442x; 1.0106x over previous
"""DiffVG-style circle renderer on 8 Trainium2 NeuronCores.

Final: per-core specialized programs via an 8-way tc.Switch on partition_id().
Each arm is fully static for its core: window widths/offsets are
compile-time constants, per-circle scalars (alpha, alpha*color) are
instruction immediates. This removes the shared cross-core max-width
padding (-25% elements), the dynamic-offset register machinery
(~140ns/slot on DVE), and the per-partition AP-scalar reads
(+60..150ns/op).

Per core: shard image by rows (128 rows/core); composite the circles
intersecting the band front-to-back (descending z) with transmittance T:
    w   = cov * T        (DVE TT 2x, cov = sigmoid(2(r-d)) from ACT/PSUM)
    T  += (-a) * w       (DVE stt, immediate scalar — chain stays on DVE)
    m_ch = (a col_ch) w  (premult, spread over ACT/Pool/DVE, immediates)
    C3  += m3            (DVE 3-plane TT, delayed 2 slots)
PE computes z=(r^2-d^2)/r per circle-pair as a K=8 outer-sum matmul with
bf16 hi/lo split operands. Output: [T->A, C_RGB] fp16 planes DMA'd out,
assembled to f32 on host.
"""

import sys

if "/opt/trn_rl_repo" not in sys.path:
    sys.path.insert(0, "/opt/trn_rl_repo")

import numpy as np
import ml_dtypes

import concourse.bass as bass
import concourse.bacc as bacc
import concourse.mybir as mybir
from concourse.tile import TileContext
from concourse import bass_utils

H = 1024
W = 1024
ROWS = 128
N_CORES = 8
MARGIN = 4.0
ROUND = 4
WMIN = 16
WCAP = 224
F32 = mybir.dt.float32
F16 = mybir.dt.float16
BF16 = mybir.dt.bfloat16
AF = mybir.ActivationFunctionType
OP = mybir.AluOpType
BF = ml_dtypes.bfloat16


# ---------------------------------------------------------------- host plan
def _core_circles(centers, radii, core):
    """Kept circle indices (descending z = front-to-back) + widths/offsets."""
    y0 = ROWS * core
    cy = centers[:, 1].astype(np.float64)
    cx = centers[:, 0].astype(np.float64)
    r = radii.astype(np.float64)
    keep = (cy + r + MARGIN >= y0 + 0.5) & (cy - r - MARGIN <= y0 + ROWS - 0.5)
    idx = np.where(keep)[0][::-1]  # descending index = front-to-back
    dymin = np.maximum(0.0, np.maximum(y0 + 0.5 - cy[idx],
                                       cy[idx] - (y0 + ROWS - 0.5)))
    rm = r[idx] + MARGIN
    halfw = np.sqrt(np.maximum(rm * rm - dymin * dymin, 4.0))
    ws = np.clip(np.ceil(2.0 * halfw / ROUND) * ROUND, WMIN, WCAP).astype(int)
    off = np.clip(np.round(cx[idx] - ws / 2.0), 0, W - ws).astype(int)
    return idx, ws, off


def _schedule(ws, off):
    """Topological reorder of the z-overlap DAG spacing each circle >=2
    slots after its last overlapping predecessor (deep-stack-first greedy).
    Returns (order, fo) where fo[k] = slot of k's first overlapping
    successor in the NEW order (n if none)."""
    n = len(ws)
    lo, hi = off, off + ws
    preds = [[] for _ in range(n)]
    succs = [[] for _ in range(n)]
    for u in range(n):
        for v in range(u + 1, n):
            if lo[u] < hi[v] and lo[v] < hi[u]:
                preds[v].append(u)
                succs[u].append(v)
    emitted_slot = {}
    emitted = np.zeros(n, bool)
    order = []
    for slot in range(n):
        ready = [v for v in range(n) if not emitted[v]
                 and all(emitted[u] for u in preds[v])]
        def lastpred(v):
            return max((emitted_slot[u] for u in preds[v]), default=-10)
        pool = ready
        for dist in (4, 3, 2):
            far = [v for v in ready if slot - lastpred(v) >= dist]
            if far:
                pool = far
                break
        if slot >= n - 12:
            v = min(pool, key=lambda v: ws[v])  # narrow tail: cheap taper
        else:
            v = max(pool, key=lambda v: (len([s for s in succs[v]
                                              if not emitted[s]]),
                                         -lastpred(v)))
        order.append(v)
        emitted[v] = True
        emitted_slot[v] = slot
    order = np.array(order, int)
    slot_of = np.empty(n, int)
    slot_of[order] = np.arange(n)
    fo = np.full(n, n, int)  # indexed by NEW slot
    lp = np.full(n, -10**6, int)  # last overlapping predecessor's slot
    root = np.zeros(n, bool)  # indexed by NEW slot
    for u in range(n):
        su = slot_of[u]
        if not preds[u]:
            root[su] = True
        for v in succs[u]:
            fo[su] = min(fo[su], slot_of[v])
            lp[slot_of[v]] = max(lp[slot_of[v]], su)
    return order, fo, root, lp


def make_plan(centers, radii, colors):
    """Per-core plan dicts with all compile-time constants."""
    col = colors.astype(np.float64)
    r = radii.astype(np.float64)
    cx = centers[:, 0].astype(np.float64)
    cy = centers[:, 1].astype(np.float64)
    plans = []
    for core in range(N_CORES):
        ids, ws, off = _core_circles(centers, radii, core)
        n = len(ids)
        order, fo, root, lp = _schedule(ws, off)
        ids, ws, off = ids[order], ws[order], off[order]
        npairs = (n + 1) // 2
        pairw = []
        for p in range(npairs):
            w0 = int(ws[2 * p])
            w1 = int(ws[2 * p + 1]) if 2 * p + 1 < n else 0
            pairw.append(w0 + w1)
        plans.append({
            "core": core, "ids": ids, "ws": ws.astype(int), "off": off,
            "n": n, "npairs": npairs, "pairw": pairw, "fo": fo, "root": root, "lp": lp,
            "alpha": col[ids, 3], "colr": col[ids, 0], "colg": col[ids, 1],
            "colb": col[ids, 2],
            "r": r[ids], "cx": cx[ids], "cy": cy[ids],
        })
    return plans


def _hilo(x):
    hi = x.astype(BF)
    lo = (x - hi.astype(np.float64)).astype(BF)
    return hi, lo


def make_inputs(plans):
    """rhs is group-strided: group g's 4 pairs are packed from DRAM offset
    g*1792 (pair_pos = within-group cumsum), so every core's group-g DMA
    reads the same DRAM range."""
    npairs_max = max(p["npairs"] for p in plans)
    ngroups_max = max((p["n"] + 7) // 8 for p in plans)
    # one DRAM tensor, group-strided: [4*ROWS lhsT chunk | 1792 rhs chunk]
    GSTR = 4 * ROWS + 1792
    ins = []
    for p in plans:
        core = p["core"]
        y0 = ROWS * core
        n = p["n"]
        op = np.zeros((8, ngroups_max * GSTR), BF)
        rows = y0 + np.arange(ROWS, dtype=np.float64) + 0.5
        for k in range(n):
            vk = int(p["ws"][k])
            off = int(p["off"][k])
            ri, cxi, cyi = p["r"][k], p["cx"][k], p["cy"][k]
            j = off + np.arange(vk, dtype=np.float64) + 0.5
            a = ri / 2.0 - (rows - cyi) ** 2 / ri
            b = ri / 2.0 - (j - cxi) ** 2 / ri
            ah, alo = _hilo(a)
            bh, blo = _hilo(b)
            pair, half = divmod(k, 2)
            rb = 4 * half
            g, ip = divmod(pair, 4)
            l0 = g * GSTR + ip * ROWS
            ls = slice(l0, l0 + ROWS)
            op[rb + 0, ls] = ah
            op[rb + 1, ls] = alo
            op[rb + 2, ls] = 1.0
            op[rb + 3, ls] = 1.0
            if g == 0:
                # group 0 sits on a fixed 224-grid (uniform across cores)
                # so its matmuls+sigmoids can run before the Switch
                c0 = g * GSTR + 4 * ROWS + (k - 8 * g) * WCAP
            else:
                c0 = g * GSTR + 4 * ROWS + sum(p["pairw"][4 * g:pair]) \
                    + (0 if half == 0 else int(p["ws"][2 * pair]))
            rs = slice(c0, c0 + vk)
            op[rb + 0, rs] = 1.0
            op[rb + 1, rs] = 1.0
            op[rb + 2, rs] = bh
            op[rb + 3, rs] = blo
        ins.append({"opnd": op})
    return ins, npairs_max, ngroups_max


# ------------------------------------------------- premult engine balancer
def _assign_premults(ws, pairw, fo, lp):
    """Greedy per-core assignment of each slot's 4 premult channels to ONE
    engine (ACT/Pool/DVE) — single producer per slot keeps the C4-add's
    cross-engine sync to one semaphore. Near slots (fused add due next
    slot) premultiply on DVE so the chain never waits cross-engine."""
    n = len(ws)
    dve = act = pool = 0.0
    for k in range(n):
        vk = float(ws[k])
        dve += (58 + vk / 2) / 0.96 + 80          # TT w'
        dve += (58 + vk) / 0.96 + 120             # stt imm (T update)
    for pw in pairw:
        act += (172 + pw) / 1.2 + 60              # sigmoid PSUM->SBUF
    c3_dve = lambda vk: (58 + 3 * vk / 2) / 0.96 + 120
    c3_pool = lambda vk: 3 * vk * 2.6 / 1.2 + 250
    cost = {
        "dve": lambda vk: 3 * ((58 + vk / 4) / 0.96 + 60),
        "act": lambda vk: 3 * ((224 + vk) / 1.2 + 60),
        "pool": lambda vk: 3 * (250 + vk * 1.2 / 1.2),
    }
    busy = {"dve": dve, "act": act, "pool": pool}
    assign = []
    for k in range(n):
        vk = float(ws[k])
        if k >= n - 12:
            e = "dve"  # tail-taper: let ACT/Pool drain
        else:
            e = min(busy, key=lambda e: (busy[e] + cost[e](vk))
                    * (1.35 if e == "dve" else 1.0))
        busy[e] += cost[e](vk)
        # C3-add: keep on DVE unless this is a Pool slot and Pool+C3 is
        # projected cheaper than loading DVE further (zero cross-engine
        # sync since the premults are already there)
        isolated = (fo[k] - k >= 8) and (k - lp[k] >= 8)
        if (e == "pool" and isolated and k < n - 14
                and busy["pool"] + c3_pool(vk) < busy["dve"] + c3_dve(vk)):
            busy["pool"] += 300.0  # SWDGE descgen on the Pool queue
            c3e = "dma"
        else:
            busy["dve"] += c3_dve(vk)
            c3e = "dve"
        assign.append((e, c3e))
    return assign


# ------------------------------------------------------------- device build
def build_nc(plans, npairs_max, ngroups_max):
    GSTR = 4 * ROWS + 1792
    nc = bacc.Bacc("TRN2", target_bir_lowering=False, debug=False,
                   num_devices=N_CORES)
    opnd_d = nc.dram_tensor("opnd", [8, ngroups_max * GSTR], BF16,
                            kind="ExternalInput").ap()
    out_d = nc.dram_tensor("out", [ROWS, 4 * W], F16,
                           kind="ExternalOutput").ap()

    with TileContext(nc) as tc:
        # TCC: plane 0 = transmittance T, planes 1-3 = premultiplied RGB.
        # One 4-plane TT add per slot applies both the T update (m_A = -a*w)
        # and the color accumulation (m_ch = a*col_ch*w).
        TCC = nc.alloc_sbuf_tensor("TCC", [ROWS, 4 * W], F16).ap()
        covr = nc.alloc_sbuf_tensor("covr", [ROWS, 3 * 1792], F16).ap()
        wr = nc.alloc_sbuf_tensor("wr", [ROWS, 8 * WCAP], F16).ap()
        mr = nc.alloc_sbuf_tensor("mr", [ROWS, 8 * 3 * WCAP], F16).ap()
        op_sb = nc.alloc_sbuf_tensor("op_sb", [8, 2 * GSTR], BF16).ap()
        pt0 = nc.alloc_psum_tensor("pt0", [ROWS, 4 * 512], F32).ap()
        pt1 = nc.alloc_psum_tensor("pt1", [ROWS, 4 * 512], F32).ap()
        pts = [pt0, pt1]

        # prefetch the first two groups' operands FIRST: the transfer takes
        # ~3.5us end-to-end and the first matmul is gated on it, so its
        # issue must not queue behind the pid-load/dispatch machinery
        cut = 4 * ROWS + 448  # pair-0 lhsT + rhs: first matmul's gate
        nc.sync.dma_start(op_sb[:, 0:cut], opnd_d[:, 0:cut])
        nc.sync.dma_start(op_sb[:, cut:GSTR], opnd_d[:, cut:GSTR])
        if ngroups_max > 1:
            nc.sync.dma_start(op_sb[:, GSTR:2 * GSTR],
                              opnd_d[:, GSTR:2 * GSTR])

        pid = nc.partition_id()
        from concourse.expressions import s_valid_engines
        hint = tc.switch_hint({e: pid for e in s_valid_engines(pid)},
                              N_CORES, label="corearm")

        T = TCC[:, 0:W]
        nc.gpsimd.memset(T, 1.0)
        nc.gpsimd.memset(TCC[:, W:4 * W], 0.0)

        TCC4 = TCC.rearrange("p (c x) -> p c x", x=W)
        CC3 = TCC[:, W:4 * W].rearrange("p (c x) -> p c x", x=W)
        mr3 = mr.rearrange("p (s x) -> p s x", x=WCAP)

        # groups 0-1 are on a fixed 224-grid on every core: run their
        # matmuls and sigmoids before the Switch, overlapping the arm
        # I-cache fill
        for g in (0,):
            h = g % 2
            for i in range(4):
                # alternate PSUM buffers so matmul i+1 never waits on
                # sigmoid i (PSUM deps track coarser than 512-col ranges)
                pb = pts[i % 2][:, (i // 2) * 1024:(i // 2) * 1024 + 448]
                nc.tensor.matmul(
                    pb,
                    op_sb[:, h * GSTR + i * ROWS:h * GSTR + (i + 1) * ROWS],
                    op_sb[:, h * GSTR + 4 * ROWS + i * 448:
                          h * GSTR + 4 * ROWS + (i + 1) * 448],
                    start=True, stop=True)
                nc.scalar.activation(
                    covr[:, (g % 3) * 1792 + i * 448:
                         (g % 3) * 1792 + (i + 1) * 448],
                    pb, AF.Sigmoid)

        for core in tc.Switch(pid, N_CORES, hint=hint):
            p = plans[core]
            n = p["n"]
            ws, off = p["ws"], p["off"]
            pairw = p["pairw"]
            fo = p["fo"]
            assign = _assign_premults(ws, pairw, fo, p["lp"])
            # C3-add for slot k can lag up to 6 slots (mr ring depth 8) —
            # only the stt updates T, so C adds never gate the chain.
            place = {}
            for k in range(n):
                j = min(k + 6, n)
                place.setdefault(j, []).append(k)

            def emit_c3(k):
                vk = int(ws[k])
                ok = int(off[k])
                qv = CC3[:, :, ok:ok + vk]
                mp = mr3[:, (k % 8) * 3:(k % 8) * 3 + 3, :vk]
                if assign[k][1] == "dma":
                    nc.gpsimd.dma_start(qv, mp, accum_op=OP.add)
                else:
                    nc.vector.tensor_tensor(qv, qv, mp, OP.add)

            stt_done = set()
            wsrc = {}

            def emit_stt(k):
                if k < 0 or k in stt_done:
                    return
                stt_done.add(k)
                vq, oq = int(ws[k]), int(off[k])
                nc.vector.scalar_tensor_tensor(
                    T[:, oq:oq + vq], wsrc[k],
                    -float(p["alpha"][k]), T[:, oq:oq + vq],
                    OP.mult, OP.add)

            ngroups = (n + 7) // 8
            for g in range(ngroups):
                k0 = g * 8
                gsize = min(8, n - k0)
                gp = (gsize + 1) // 2
                p0 = k0 // 2
                lh = op_sb[:, (g % 2) * GSTR:]
                rh = op_sb[:, (g % 2) * GSTR + 4 * ROWS:]
                if g >= 2:
                    nc.sync.dma_start(
                        op_sb[:, (g % 2) * GSTR:(g % 2 + 1) * GSTR],
                        opnd_d[:, g * GSTR:(g + 1) * GSTR])
                pt = pts[g % 2]
                cbase = (g % 3) * 1792
                rpos = 0
                pair_pos = []
                if g > 0:
                    for i in range(gp):
                        pw = pairw[p0 + i]
                        pair_pos.append(rpos)
                        nc.tensor.matmul(
                            pt[:, i * 512:i * 512 + pw],
                            lh[:, i * ROWS:(i + 1) * ROWS],
                            rh[:, rpos:rpos + pw],
                            start=True, stop=True)
                        rpos += pw

                rpos = 0
                for j in range(gsize):
                    k = k0 + j
                    vk = int(ws[k])
                    ok = int(off[k])
                    if g == 0:
                        rpos = j * WCAP  # fixed grid, sigmoid pre-dispatch
                    elif j % 2 == 0:
                        i = j // 2
                        pw = pairw[p0 + i]
                        nc.scalar.activation(
                            covr[:, cbase + pair_pos[i]:
                                 cbase + pair_pos[i] + pw],
                            pt[:, i * 512:i * 512 + pw], AF.Sigmoid)
                    cov = covr[:, cbase + rpos:cbase + rpos + vk]
                    rpos += vk
                    wv = wr[:, (k % 8) * WCAP:(k % 8) * WCAP + vk]
                    tw = T[:, ok:ok + vk]
                    al = float(p["alpha"][k])
                    acr = float(p["alpha"][k] * p["colr"][k])
                    acg = float(p["alpha"][k] * p["colg"][k])
                    acb = float(p["alpha"][k] * p["colb"][k])
                    # DVE: w = cov*T, then lagged C add, then the PREVIOUS
                    # slot's T update — the one-slot stt lag keeps every
                    # same-window DVE dependency >=3 ops apart so Tile's
                    # same-engine completion waits (~600ns) never fire.
                    # Safe: the scheduler spaces overlapping windows >=2
                    # slots, so slot k+1's w' never needs slot k's update.
                    # Root slots (no overlapping higher-z circle) see T==1:
                    # w == cov, so the w'-TT is skipped entirely.
                    if p["root"][k]:
                        wsrc[k] = cov
                    else:
                        wsrc[k] = wv
                        nc.vector.tensor_tensor(wv, cov, tw, OP.mult)
                    for kp in place.get(k, ()):
                        emit_c3(kp)
                    emit_stt(k - 1)
                    if int(fo[k]) <= k + 1:
                        # next slot overlaps: T update cannot be delayed
                        emit_stt(k)
                    # premults m_ch = (a col_ch) * w, one engine per slot
                    eng = assign[k][0]
                    wk = wsrc[k]
                    for ch, sc in enumerate((acr, acg, acb)):
                        mb = ((k % 8) * 3 + ch) * WCAP
                        if eng == "dve":
                            nc.vector.tensor_scalar(
                                mr[:, mb:mb + vk], wk, sc, 0.0,
                                OP.mult, OP.add)
                        elif eng == "pool":
                            nc.gpsimd.tensor_scalar(
                                mr[:, mb:mb + vk], wk, sc, 0.0,
                                OP.mult, OP.add)
                        else:
                            nc.scalar.activation(
                                mr[:, mb:mb + vk], wk, AF.Copy, scale=sc)
            emit_stt(n - 1)
            # ship TCC raw ([T|R|G|B]); the host computes A = 1 - T.
            # chunk bounds cover the flush C-add windows AND slot n-1's
            # T window (the only T write emitted after the early DMAs)
            flushks = sorted(kk for j, ks in place.items() if j >= n
                             for kk in ks)
            xlo = min((int(off[k]) for k in flushks + [n - 1]))
            xhi = max((int(off[k]) + int(ws[k]) for k in flushks + [n - 1]))
            out4 = out_d.rearrange("p (c x) -> p c x", x=W)
            src4 = TCC.rearrange("p (c x) -> p c x", x=W)
            if xlo > 0:
                nc.sync.dma_start(out4[:, :, 0:xlo], src4[:, :, 0:xlo])
            if xhi < W:
                nc.sync.dma_start(out4[:, :, xhi:W], src4[:, :, xhi:W])
            for k in flushks:
                emit_c3(k)
            if xhi > xlo:
                nc.sync.dma_start(out4[:, :, xlo:xhi], src4[:, :, xlo:xhi])


    nc.compile()
    return nc


def kernel(centers, radii, colors):
    centers = np.asarray(centers, np.float32)
    radii = np.asarray(radii, np.float32)
    colors = np.asarray(colors, np.float32)

    plans = make_plan(centers, radii, colors)
    ins, npairs_max, ngroups_max = make_inputs(plans)
    nc = build_nc(plans, npairs_max, ngroups_max)
    res = bass_utils.run_bass_kernel_spmd(nc, ins, list(range(N_CORES)),
                                          trace=False)
    out = np.empty((H, W, 4), np.float32)
    for c in range(N_CORES):
        planes = res.results[c]["out"].astype(np.float32)  # [128, 4*W]
        out[c * ROWS:(c + 1) * ROWS, :, 3] = 1.0 - planes[:, 0:W]
        for ch in range(3):
            out[c * ROWS:(c + 1) * ROWS, :, ch] = \
                planes[:, (ch + 1) * W:(ch + 2) * W]
    return out
